# revision 72
# baseline (speedup 1.0000x reference)
"""Trainium2 Bass kernel for nn_Net_66451734004145 (GRU -> "adjacency" ->
MLP -> log_softmax over the S*S pair dim).

Key structural fact: the reference's adjacency reshape (faithful torch
translation) scrambles the pairwise concat.  For p = i*S + j:
    j <  S/2 : row = [y_i, y_i]            (depends only on i)
    j >= S/2 : row = [y_{2j-S}, y_{2j-S+1}] (depends only on j)
So the MLP has only S + S/2 = 192 distinct rows per batch element: 128
"A" rows (one per i) and 64 "B" rows (one per j-64).  The dim-0
log_softmax over all S*S rows reduces to
    lse = log(64*sum_i exp(lgA_i) + 128*sum_j exp(lgB_j))
and bt cancels (constant along dim 0).  The kernel computes the GRU (the
dominant, latency-bound part: 128 sequential steps), the 192-row MLP, the
weighted lse, and expands the output via broadcast DMAs.

Sharding: data-parallel over batch B=16 across 8 cores (2 per core); the
log_softmax dim stays local, no collectives.

The two per-core batch elements run as two independent software-pipelined
GRU chains (per-(t,b) scheduler floors keep the engine queues interleaved),
each with cycle: [MM r,z] -> sigmoid -> tanh(scale=r, bias=gin) ->
fused h' = n*z' + (h - z'*h) (one scalar_tensor_tensor). All state
columns are duplicated (cols 2t, 2t+1) so every fp32r matmul sees an
N=2 rhs at an even offset. The 192-row MLP tail merges the weighted lse
into one exp+accum per batch via a +ln2 bias row on B logits, keeps exp
and ln in one activation-table set, and expands the (i,j) grid with K=65
selector matmuls (all PE operands at base partition 0).

Output NEFF layout per core: [128, 4, 128] f32 = [i, (b,f), j]; host
transposes to (S*S, 2, 2) and concatenates over cores along batch.
"""

import contextlib
import math

import numpy as np

import concourse.bass as bass
import concourse.mybir as mybir
import concourse.tile as tile
from concourse import bacc
from concourse.bass import ds, ts
from concourse.bass_utils import run_bass_kernel_spmd

S = 128
B = 16
IN = 64
H = 100
HID = 256
NCORES = 8
BL = B // NCORES  # 2
NR = S + S // 2  # 192 distinct MLP rows per batch element

F32 = mybir.dt.float32
F32R = mybir.dt.float32r
AF = mybir.ActivationFunctionType
ALU = mybir.AluOpType

# blob packing: name -> (rows, cols); column offsets are cumulative.
# hot blobs land first (GRU-critical), cold holds everything the MLP tail
# needs; split across DMA queues so completion isn't serialized.
_BLOB_WHH_LAYOUT = [          # f32r, sync queue (GRU h-weights)
    ("whh", H + 1, 3 * H),
]
_BLOB_WIH_LAYOUT = [          # f32r, scalar queue (GRU x-weights + h0)
    ("wih", IN + 1, 3 * H),
    ("h0c", H, 2 * BL),       # initial hidden, duplicated [b0,b0,b1,b1]
]
_BLOB_XT_LAYOUT = [           # f32r, gpsimd queue (time-half A: t < 64)
    ("xta", IN + 1, S * BL),      # per-chain blocks, columns duplicated
]
_BLOB_XTB_LAYOUT = [          # f32r, gpsimd queue (time-half B: t >= 64)
    ("xtb", IN + 1, S * BL),
]
_BLOB_COLD_LAYOUT = [         # f32r, vector queue (MLP weights)
    ("w1ab", H + 1, HID),
    ("w1a", H + 1, HID),
    ("w1b", H + 1, HID),
    ("w2", 128, 512),
    ("w3", 128, 20),
    ("wt", 65, 65),           # f0 -> col 0, f1 -> col 64 (planes at legal
                              # base partitions); row 64: +ln2 on B logits
    # broadcast-expansion selectors (K=65, all operands base partition 0):
    ("sela", 65, 128),        # row0 = 1[0:64], row64 = 1[64:128]
    ("pick0", 65, 128),       # row0 = ones
    ("pick64", 65, 128),      # row64 = ones
]
_BLOB_F_LAYOUT = [            # f32, scalar queue (non-PE operands)
    ("b2v", 128, 2),
    ("b3c", 10, 1),
    ("h3ind", 1, 2 * NR),     # B-region indicator row for the h3 aug
]


def _offsets(layout):
    off, o = {}, 0
    for name, _r, c in layout:
        off[name] = o
        o += c
    return off, o


BLOB_WHH_OFF, C_WHH = _offsets(_BLOB_WHH_LAYOUT)
BLOB_WIH_OFF, C_WIH = _offsets(_BLOB_WIH_LAYOUT)
BLOB_XT_OFF, C_XT = _offsets(_BLOB_XT_LAYOUT)
BLOB_XTB_OFF, C_XTB = _offsets(_BLOB_XTB_LAYOUT)
BLOB_COLD_OFF, C_COLD = _offsets(_BLOB_COLD_LAYOUT)
BLOB_F_OFF, C_F = _offsets(_BLOB_F_LAYOUT)


def bcast_free(ap, n, axis):
    """Insert a broadcast (step 0, count n) free dim at free-axis position."""
    newap = [list(d) for d in ap.ap]
    newap.insert(1 + axis, [0, n])
    return bass.AP(tensor=ap.tensor, offset=ap.offset, ap=newap)


def _emit(nc, tc):
    # ---------------- DRAM I/O ----------------
    bwhh = nc.dram_tensor("bwhh", [128, C_WHH], F32R, kind="ExternalInput").ap()
    bwih = nc.dram_tensor("bwih", [128, C_WIH], F32R, kind="ExternalInput").ap()
    bxta = nc.dram_tensor("bxta", [128, C_XT], F32R, kind="ExternalInput").ap()
    bxtb = nc.dram_tensor("bxtb", [128, C_XTB], F32R, kind="ExternalInput").ap()
    bcold = nc.dram_tensor("bcold", [128, C_COLD], F32R, kind="ExternalInput").ap()
    bf = nc.dram_tensor("bf", [128, C_F], F32, kind="ExternalInput").ap()
    out_d = nc.dram_tensor("out", [S, 2 * BL, S], F32, kind="ExternalOutput").ap()

    with contextlib.ExitStack() as ctx:
        consts = ctx.enter_context(tc.tile_pool(name="consts", bufs=1))
        singles = ctx.enter_context(tc.tile_pool(name="singles", bufs=1))

        t_whh = consts.tile([128, C_WHH], F32R, tag="bwhh")
        nc.sync.dma_start(out=t_whh[:], in_=bwhh)
        t_wih = consts.tile([128, C_WIH], F32R, tag="bwih")
        nc.scalar.dma_start(out=t_wih[:], in_=bwih)
        t_xt = consts.tile([128, C_XT], F32R, tag="bxta")
        nc.gpsimd.dma_start(out=t_xt[:], in_=bxta)
        t_xtb = consts.tile([128, C_XTB], F32R, tag="bxtb")
        nc.gpsimd.dma_start(out=t_xtb[:], in_=bxtb)
        t_cold = consts.tile([128, C_COLD], F32R, tag="bcold")
        nc.gpsimd.dma_start(out=t_cold[:], in_=bcold)
        t_f = consts.tile([128, C_F], F32, tag="bf")
        nc.gpsimd.dma_start(out=t_f[:], in_=bf)

        # activation-table warmup: leave the sigmoid/tanh set resident
        # before the GRU's first step (one table load, after the DMA
        # descriptor-gen so it doesn't delay the weight transfers).
        wu = singles.tile([1, 4], F32)
        nc.vector.memset(wu[:, :], 1.0)
        nc.scalar.activation(wu[:, 0:1], wu[:, 1:2], AF.Sigmoid)

        def sl(tileap, offs, name, rows, cols):
            return tileap[0:rows, ds(offs[name], cols)]

        # Y built on-chip: ones everywhere (aug row + to-be-overwritten h
        # region), h_init columns copied from the wih blob (f32r, no cast).
        Y = singles.tile([H + 1, BL * 2 * (S + 1)], F32R)
        nc.vector.memset(Y[:, :].bitcast(F32), 1.0)
        h0c_s = sl(t_wih, BLOB_WIH_OFF, "h0c", H, 2 * BL)
        for b in range(BL):
            nc.vector.tensor_copy(
                Y[0:H, ds(b * 2 * (S + 1), 2)],
                h0c_s[:, ds(2 * b, 2)],
            )

        whh_s = sl(t_whh, BLOB_WHH_OFF, "whh", H + 1, 3 * H)
        wih_s = sl(t_wih, BLOB_WIH_OFF, "wih", IN + 1, 3 * H)
        whh_g = [whh_s[:, ts(g, H)] for g in range(3)]
        wih_g = [wih_s[:, ts(g, H)] for g in range(3)]
        xt_s = sl(t_xt, BLOB_XT_OFF, "xta", IN + 1, S * BL)
        xtb_s = sl(t_xtb, BLOB_XTB_OFF, "xtb", IN + 1, S * BL)
        w1ab_s = sl(t_cold, BLOB_COLD_OFF, "w1ab", H + 1, HID)
        w1a_s = sl(t_cold, BLOB_COLD_OFF, "w1a", H + 1, HID)
        w1b_s = sl(t_cold, BLOB_COLD_OFF, "w1b", H + 1, HID)
        w2_s = sl(t_cold, BLOB_COLD_OFF, "w2", 128, 512).rearrange(
            "p (a b c) -> p a b c", a=2, b=2
        )
        w3_s = sl(t_cold, BLOB_COLD_OFF, "w3", 128, 20).rearrange(
            "p (a c) -> p a c", a=2
        )
        wt_s = sl(t_cold, BLOB_COLD_OFF, "wt", 65, 65)
        sela_s = sl(t_cold, BLOB_COLD_OFF, "sela", 65, 128)
        pick0_s = sl(t_cold, BLOB_COLD_OFF, "pick0", 65, 128)
        pick64_s = sl(t_cold, BLOB_COLD_OFF, "pick64", 65, 128)
        b2v_s = sl(t_f, BLOB_F_OFF, "b2v", 128, 2)
        b3c_s = sl(t_f, BLOB_F_OFF, "b3c", 10, 1)
        h3ind_s = sl(t_f, BLOB_F_OFF, "h3ind", 1, 2 * NR)

        # Y holds, per chain b, [h_{-1}, h_0, ..., h_{127}] feature-major
        # with an aug ones row; every column is DUPLICATED (cols 2t, 2t+1
        # both hold h_{t-1}) so all fp32r matmuls see N=2 rhs at even
        # offsets (the ISA rejects N=1 fp32r matmuls).
        GIN = singles.tile([H, 2 * S * BL], F32)

        # ---------------- GRU: two independent per-batch chains -----------
        # The two batch elements' recurrences are independent; emitting
        # their per-step ops interleaved (b0 block, b1 block) lets the two
        # dependency chains pipeline across engines, so each chain's cycle
        # is its own latency (~1.55us) instead of sharing one serialized
        # joint-chain cycle (~1.94us).
        with contextlib.ExitStack() as gru_ctx:
            pgi = gru_ctx.enter_context(tc.tile_pool(name="pgi", bufs=1, space="PSUM"))
            pghn = [
                gru_ctx.enter_context(
                    tc.tile_pool(name=f"pghn{b}", bufs=2, space="PSUM")
                )
                for b in range(BL)
            ]
            rings = [
                gru_ctx.enter_context(tc.tile_pool(name=f"rings{b}", bufs=3))
                for b in range(BL)
            ]

            # PSUM start=True lazily zeroes a whole 2KB bank (zero region):
            # only the first matmul touching each bank may use start=True.
            # Per-chain layout [100, 3, 2S] (3KB -> two banks per chain):
            # g=0 (start) zeroes bank0 (covers g=0,1), g=2 (start) zeroes
            # bank1; per-step gh matmuls accumulate into written bytes.
            # Cell: h' = z'*n + (h - z'*h) with z' = sigmoid(-(i_z + h_z))
            # (z-gate weights negated on host), so no z gate is computed.
            psum_gi = [
                pgi.tile([H, 3, 2 * S], F32, tag=f"pgi{b}", name=f"psum_gi{b}")
                for b in range(BL)
            ]
            Yb = [Y[:, ds(b * 2 * (S + 1), 2 * (S + 1))] for b in range(BL)]
            GINb = [GIN[:, ds(b * 2 * S, 2 * S)] for b in range(BL)]

            # gi precompute in time-halves: the GRU's first steps only wait
            # for the (smaller, earlier) first-half xt DMA.
            for b in range(BL):
                for g in range(3):
                    nc.tensor.matmul(
                        psum_gi[b][:, g, ds(0, S)],
                        lhsT=wih_g[g],
                        rhs=xt_s[:, ds(b * S, S)],
                        start=(g != 1),
                        stop=False,
                        skip_group_check=True,
                    )
                nc.scalar.activation(
                    GINb[b][:, ds(0, S)], psum_gi[b][:, 2, ds(0, S)], AF.Copy
                )
            # floored into mid-GRU engine-idle gaps so the ramp (first
            # steps) isn't delayed; deps still guard the t>=64 accumulates
            with tc.tile_wait_until(0.080):
                for b in range(BL):
                    for g in range(3):
                        nc.tensor.matmul(
                            psum_gi[b][:, g, ds(S, S)],
                            lhsT=wih_g[g],
                            rhs=xtb_s[:, ds(b * S, S)],
                            start=False,
                            stop=False,
                            skip_group_check=True,
                        )
                    nc.scalar.activation(
                        GINb[b][:, ds(S, S)], psum_gi[b][:, 2, ds(S, S)],
                        AF.Copy,
                    )

            # Per-(t, b) logical-time floors force the scheduler to keep the
            # two chains' instructions interleaved (b0 t, b1 t, b0 t+1, ...)
            # in every engine queue; without them the greedy list scheduler
            # reorders (e.g. sig_b1 ahead of tanh_b0), coupling the chains.
            # The floors only shape queue ORDER; hardware runs on semaphores.
            for t in range(S):
                for b in range(BL):
                    ctx_wait = tc.tile_wait_until(0.013 + (2 * t + b) * 0.001)
                    ctx_wait.__enter__()
                    hcols = Yb[b][:, ds(2 * t, 2)]
                    for g in range(2):
                        nc.tensor.matmul(
                            psum_gi[b][:, g, ds(2 * t, 2)],
                            lhsT=whh_g[g],
                            rhs=hcols,
                            start=False,
                            stop=True,
                            skip_group_check=True,
                        )
                    ghn = pghn[b].tile([H, 2], F32, tag=f"ghn{b}")
                    nc.tensor.matmul(
                        ghn[:], lhsT=whh_g[2], rhs=hcols,
                        start=True, stop=True,
                    )
                    rzp = rings[b].tile([H, 2], F32, tag=f"rzp{b}")
                    nc.scalar.activation(
                        rzp[:],
                        psum_gi[b][:, 0:2, ds(2 * t, 1)].rearrange(
                            "p a c -> p (a c)"
                        ),
                        AF.Sigmoid,
                    )
                    ng = rings[b].tile([H, 1], F32, tag=f"ng{b}")
                    nc.scalar.activation(
                        ng[:], ghn[:, ds(0, 1)], AF.Tanh,
                        scale=rzp[:, ds(0, 1)],
                        bias=GINb[b][:, ds(2 * t, 1)],
                    )
                    # u = h - z'*h in the tanh's shadow; fused
                    # h' = n*z' + u is the only post-tanh chain op (it
                    # writes both duplicate columns via 0-stride reads).
                    vv = rings[b].tile([H, 1], F32, tag=f"vv{b}")
                    uu = rings[b].tile([H, 1], F32, tag=f"uu{b}")
                    hold = Yb[b][0:H, ds(2 * t, 1)].bitcast(F32)
                    nc.vector.tensor_mul(vv[:], hold, rzp[:, ds(1, 1)])
                    nc.vector.tensor_sub(uu[:], hold, vv[:])
                    nc.vector.scalar_tensor_tensor(
                        Yb[b][0:H, ds(2 * (t + 1), 2)].rearrange(
                            "p (a c) -> p a c", c=1
                        ),
                        bcast_free(ng[:], 2, 0),
                        rzp[:, ds(1, 1)],
                        bcast_free(uu[:], 2, 0),
                        op0=ALU.mult,
                        op1=ALU.add,
                    )
                    ctx_wait.__exit__(None, None, None)

        # ---------------- 192-row MLP + lse + output expansion ------------
        # column views of Y per batch (duplicated cols; pick duplicate 0):
        # yb_v = all y_t, y4_v[:, 0/2, :] = even/odd y_t
        yb_v = [
            Y[:, ds(b * 2 * (S + 1) + 2, 2 * S)].rearrange(
                "p (t two) -> p two t", two=2
            )[:, 0, :]
            for b in range(BL)
        ]
        y4_v = [
            Y[:, ds(b * 2 * (S + 1) + 2, 2 * S)].rearrange(
                "p (k f) -> p f k", f=4
            )
            for b in range(BL)
        ]
        # y4_v[b][:, 2k, :] == y_{2j+k} columns for batch b

        import os
        if os.environ.get("DBG_SKIP_TAIL"):
            sbz = singles.tile([S, 2 * BL * S], F32)
            nc.vector.memset(sbz[:, :], 0.0)
            nc.vector.tensor_copy(sbz[0:H, 0:4], Y[0:H, 0:4].bitcast(F32))
            nc.sync.dma_start(
                out=out_d.rearrange("a b c -> a (b c)"), in_=sbz[:, :]
            )
            return

        # prewarm the Exp table during the W1/W2 matmuls: the dummy exp
        # depends on the b1 chain's final h column so its table load lands
        # right after the GRU (never mid-loop), hidden under the W1 MMs.
        nc.scalar.activation(
            wu[:, 2:3],
            Y[0:1, ds(2 * (S + 1) + 2 * S, 1)].bitcast(F32),
            AF.Exp,
        )

        with contextlib.ExitStack() as mlp_ctx:
            pmm = mlp_ctx.enter_context(tc.tile_pool(name="pmm", bufs=1, space="PSUM"))

            # [p, fc, b, row]; bank0 = cols 0:512, bank1 = 512:768.  start=True
            # only on each bank's first matmul in program order (zero-region
            # semantics); everything else relies on pending-zero overwrite /
            # accumulate-on-written-bytes.
            psAB = pmm.tile([128, 2, 2, NR], F32)
            for b in range(BL):
                for fc in range(2):
                    nc.tensor.matmul(
                        psAB[:, fc, b, ds(0, S)],
                        lhsT=w1ab_s[:, ts(fc, 128)],
                        rhs=yb_v[b],
                        start=(b == 0 and fc == 0), stop=False,
                        skip_group_check=True,
                    )
                    nc.tensor.matmul(
                        psAB[:, fc, b, ds(S, S // 2)],
                        lhsT=w1a_s[:, ts(fc, 128)],
                        rhs=y4_v[b][:, 0, :],
                        start=(b == 0 and fc == 1), stop=False,
                        skip_group_check=True,
                    )
                    nc.tensor.matmul(
                        psAB[:, fc, b, ds(S, S // 2)],
                        lhsT=w1b_s[:, ts(fc, 128)],
                        rhs=y4_v[b][:, 2, :],
                        start=False, stop=(b == 1),
                        skip_group_check=True,
                    )
            h1 = singles.tile([128, 2, 2 * NR], F32R)
            nc.scalar.activation(
                h1[:, 0, :],
                psAB[:, 0, :, :].rearrange("p b c -> p (b c)"),
                AF.Relu,
            )
            nc.vector.tensor_scalar_max(
                h1[:, 1, :],
                psAB[:, 1, :, :].rearrange("p b c -> p (b c)"),
                0.0,
            )

            # mc stride padded to 512 so each matmul output stays in one bank
            ps2 = pmm.tile([128, 2, 512], F32)
            for mc in range(2):
                for kc in range(2):
                    nc.tensor.matmul(
                        ps2[:, mc, ds(0, 2 * NR)],
                        lhsT=w2_s[:, kc, mc, :],
                        rhs=h1[:, kc, :],
                        start=(kc == 0),
                        stop=(kc == 1),
                    )
            h2 = singles.tile([128, 2, 2 * NR], F32R)
            nc.scalar.activation(
                h2[:, 0, :], ps2[:, 0, ds(0, 2 * NR)], AF.Relu,
                bias=b2v_s[:, ds(0, 1)],
            )
            nc.vector.tensor_scalar(
                h2[:, 1, :], ps2[:, 1, ds(0, 2 * NR)],
                b2v_s[:, ds(1, 1)], 0.0, op0=ALU.add, op1=ALU.max,
            )

            # h3 gets an aug row: 0 for A-region cols, 1 for B-region cols,
            # so the Wt matmul's 11th weight row (+ln2) lands only on B
            # logits; exp then counts B rows 2x and one exp+accum per batch
            # yields s = sum_A exp + 2*sum_B exp directly.
            ps3 = pmm.tile([10, 2 * NR], F32)
            for kc in range(2):
                nc.tensor.matmul(
                    ps3[:], lhsT=w3_s[:, kc, :], rhs=h2[:, kc, :],
                    start=(kc == 0), stop=(kc == 1),
                )
            h3 = singles.tile([65, 2 * NR], F32R)
            nc.vector.memset(h3[:, :].bitcast(F32), 0.0)
            nc.vector.tensor_copy(h3[ds(64, 1), :], h3ind_s)
            nc.vector.tensor_scalar(
                h3[0:10, :], ps3[:], b3c_s[:, ds(0, 1)], 0.0,
                op0=ALU.add, op1=ALU.max,
            )

            if os.environ.get("DBG_TAIL_STAGE") == "1":
                sbz = singles.tile([S, 2 * BL * S], F32)
                nc.vector.memset(sbz[:, :], 0.0)
                nc.vector.tensor_copy(sbz[0:65, 0:4], h3[:, 0:4].bitcast(F32))
                nc.sync.dma_start(
                    out=out_d.rearrange("a b c -> a (b c)"), in_=sbz[:, :]
                )
                return

            # logits [f-plane, (b, row)], f0 on partition 0, f1 on partition
            # 32 (matmul operands need base partition 0/32/64); B cols +ln2
            ps4 = pmm.tile([65, 2 * NR], F32)
            for b in range(BL):
                nc.tensor.matmul(
                    ps4[:, ds(b * NR, NR)], lhsT=wt_s[:],
                    rhs=h3[:, ds(b * NR, NR)],
                    start=(b == 0), stop=(b == 1),
                    skip_group_check=True,
                )

            # weighted lse over dim 0: lse = ln(64*(sum_A exp + 2*sum_B exp))
            sfull = singles.tile([65, BL], F32)
            scr = singles.tile([65, 2 * NR], F32)
            for b in range(BL):
                nc.scalar.activation(
                    scr[:, ds(b * NR, NR)], ps4[:, ds(b * NR, NR)], AF.Exp,
                    accum_out=sfull[:, ds(b, 1)],
                )
            lse = singles.tile([65, BL], F32)
            nc.scalar.activation(lse[:], sfull[:], AF.Ln, scale=64.0)
            # nlseA = -lse (A region); nlseB = -lse - ln2 (cancels the +ln2)
            nlseA = singles.tile([65, BL], F32)
            nc.vector.tensor_scalar_mul(nlseA[:], lse[:], -1.0)
            nlseB = singles.tile([65, BL], F32)
            nc.vector.tensor_scalar_sub(nlseB[:], nlseA[:], float(np.log(2.0)))

            # lg[f-plane, b, row] = final log-softmax values for the 192 rows
            lg = singles.tile([65, BL, NR], F32R)
            for b in range(BL):
                nc.vector.tensor_scalar_add(
                    lg[:, b, ds(0, S)], ps4[:, ds(b * NR, S)], nlseA[:, ds(b, 1)]
                )
                nc.vector.tensor_scalar_add(
                    lg[:, b, ds(S, S // 2)], ps4[:, ds(b * NR + S, S // 2)],
                    nlseB[:, ds(b, 1)],
                )

            if os.environ.get("DBG_TAIL_STAGE") == "3":
                sbz = singles.tile([S, 2 * BL * S], F32)
                nc.vector.memset(sbz[:, :], 0.0)
                nc.vector.tensor_copy(sbz[0:65, 0:4], lg[:, 0, 0:4].bitcast(F32))
                nc.sync.dma_start(
                    out=out_d.rearrange("a b c -> a (b c)"), in_=sbz[:, :]
                )
                return

            # Expand to the (i, j) grid with K=65 selector matmuls, all
            # operands at base partition 0 (non-zero PE base partitions die
            # on hardware). Regions per b: [A-f0 | A-f1 | B-f0 | B-f1]:
            #   A (both f): lhsT = lg A-rows [65,128], rhs = sela
            #   B-f:        lhsT = pick-f,             rhs = lg B-rows
            pout = pmm.tile([128, BL, 4, S // 2], F32)
            for b in range(BL):
                nc.tensor.matmul(
                    pout[:, b, ds(0, 2), :].rearrange("p a c -> p (a c)"),
                    lhsT=lg[:, b, ds(0, S)],
                    rhs=sela_s,
                    start=(b == 0), stop=False,
                    skip_group_check=True,
                )
                for fo in range(2):
                    nc.tensor.matmul(
                        pout[:, b, 2 + fo, :],
                        lhsT=(pick0_s if fo == 0 else pick64_s),
                        rhs=lg[:, b, ds(S, S // 2)],
                        start=False, stop=(b == 1 and fo == 1),
                        skip_group_check=True,
                    )
            if os.environ.get("DBG_TAIL_STAGE") == "6":
                sbz = singles.tile([S, 2 * BL * S], F32)
                nc.vector.memset(sbz[:, :], 0.0)
                nc.vector.tensor_copy(
                    sbz[:, 0:4], pout.rearrange("p a b c -> p (a b c)")[:, 0:4]
                )
                nc.sync.dma_start(
                    out=out_d.rearrange("a b c -> a (b c)"), in_=sbz[:, :]
                )
                return
            # stage to SBUF per-b so b0's DMA launches while b1 copies;
            # one DMA per b moves both f-planes (regions (A-fo, B-fo) sit
            # at stride 2 in sbout's dim 2)
            # the copy permutes regions (A0,A1,B0,B1) -> (A0,B0,A1,B1) so
            # each per-b DMA is a plain contiguous 1KB-per-partition copy
            sbout = singles.tile([128, BL, 2, 2, S // 2], F32)
            engs = [nc.sync, nc.scalar]
            for b in range(BL):
                nc.vector.tensor_copy(
                    sbout[:, b, :, :, :],
                    pout[:, b, :, :].rearrange("p (x y) c -> p y x c", x=2),
                )
                engs[b].dma_start(
                    out=out_d[:, ds(2 * b, 2), :],
                    in_=sbout[:, b, :, :, :].rearrange("p y x c -> p (y x c)"),
                )

        import os
        if os.environ.get("KERNEL_DEBUG_Y"):
            ydbg = nc.dram_tensor(
                "ydbg", [H + 1, BL * 2 * (S + 1)], F32, kind="ExternalOutput"
            ).ap()
            nc.sync.dma_start(out=ydbg, in_=Y[:, :])


def _patch_act_tables():
    """Steer the act-table first-fit so Exp and Ln resolve to the
    natural_log_exp_and_others set: the tail then needs ONE table load
    (hidden under the W1 matmuls) instead of two serial ones. Mutates the
    cached table dict in place (set IDs are positional and must not move;
    dropping a function from an earlier set only changes first-fit)."""
    from concourse.hw_specs import get_activation_tables

    tabs = get_activation_tables("gen3")
    for name, t in list(tabs.items()):
        if name != "natural_log_exp_and_others":
            t.discard(AF.Exp)
            if name != "sigmoid_and_others":
                t.discard(AF.Ln)


def build_nc():
    try:
        _patch_act_tables()
    except Exception:
        pass
    nc = bacc.Bacc(
        "TRN2",
        target_bir_lowering=False,
        debug=False,
        enable_asserts=False,
        num_devices=NCORES,
    )
    with tile.TileContext(nc) as tc:
        _emit(nc, tc)
    nc.compile()
    return nc


def _sel_consts():
    sela = np.zeros((65, 128), np.float32)
    sela[0, 0:64] = 1.0
    sela[64, 64:128] = 1.0
    pick0 = np.zeros((65, 128), np.float32)
    pick0[0, :] = 1.0
    pick64 = np.zeros((65, 128), np.float32)
    pick64[64, :] = 1.0
    return sela, pick0, pick64


def _wt_padded(Wt):
    # [65, 65]: f0 weights in col 0, f1 in col 64; row 64 = +ln2 (B bias)
    wt = np.zeros((65, 65), np.float32)
    wt[0:10, 0] = Wt[0]
    wt[0:10, 64] = Wt[1]
    wt[64, 0] = wt[64, 64] = np.log(2.0)
    return wt


def prep_weights(W_ih, W_hh, b_ih, b_hh, W1, b1, W2, b2, W3, b3, Wt, bt):
    """Host-side weight preprocessing shared by all cores."""
    f = np.float32
    W_ih, W_hh = f(W_ih), f(W_hh)
    b_ih, b_hh = f(b_ih), f(b_hh)
    W1, b1, W2, b2 = f(W1), f(b1), f(W2), f(b2)
    W3, b3, Wt = f(W3), f(b3), f(Wt)

    def gate(W, bvec, g, sign=1.0):
        blk = np.concatenate(
            [W[g * H : (g + 1) * H].T, bvec[g * H : (g + 1) * H][None, :]], axis=0
        )
        return sign * blk

    # gate blocks [r, z'(= -z), n]: z' weights negated so sigmoid gives 1-z
    whh = np.concatenate(
        [gate(W_hh, b_hh, 0), gate(W_hh, b_hh, 1, -1.0), gate(W_hh, b_hh, 2)],
        axis=1,
    )
    wih = np.concatenate(
        [gate(W_ih, b_ih, 0), gate(W_ih, b_ih, 1, -1.0), gate(W_ih, b_ih, 2)],
        axis=1,
    )
    W1a, W1b = W1[:, :H], W1[:, H:]
    zrow = np.zeros((1, HID), np.float32)
    # B-region indicator row of the h3 aug: 0 for A cols, 1 for B cols
    h3ind = np.tile(
        np.concatenate([np.zeros(S, np.float32), np.ones(S // 2, np.float32)]),
        BL,
    )[None, :]
    sela, pick0, pick64 = _sel_consts()
    parts = {
        "whh": whh,
        "sela": sela,
        "pick0": pick0,
        "pick64": pick64,
        "wih": wih,
        "h0c": np.zeros((H, 2 * BL), np.float32),  # filled per-core
        "w1ab": np.concatenate([(W1a + W1b).T, b1[None, :]], axis=0),
        "w1a": np.concatenate([W1a.T, b1[None, :]], axis=0),
        "w1b": np.concatenate([W1b.T, zrow], axis=0),
        "w2": W2.reshape(2, 128, 2, 128).transpose(3, 2, 0, 1).reshape(128, 512),
        "b2v": b2.reshape(2, 128).T,
        "w3": W3.reshape(10, 2, 128).transpose(2, 1, 0).reshape(128, 20),
        "b3c": b3[:, None],
        "wt": _wt_padded(Wt),

        "h3ind": h3ind,
    }

    def build(layout, offs, width, rows_total):
        blob = np.zeros((rows_total, width), np.float32)
        for name, rows, cols in layout:
            a = np.asarray(parts[name], np.float32)
            assert a.shape == (rows, cols), (name, a.shape, rows, cols)
            blob[0:rows, offs[name] : offs[name] + cols] = a
        return blob

    return {
        "bwhh": build(_BLOB_WHH_LAYOUT, BLOB_WHH_OFF, C_WHH, 128),
        "bwih": build(_BLOB_WIH_LAYOUT, BLOB_WIH_OFF, C_WIH, 128),
        "bcold": build(_BLOB_COLD_LAYOUT, BLOB_COLD_OFF, C_COLD, 128),
        "bf": build(_BLOB_F_LAYOUT, BLOB_F_OFF, C_F, 128),
    }


def make_in_maps(x, hidden, weights):
    x = np.asarray(x, np.float32)
    hidden = np.asarray(hidden, np.float32)
    in_maps = []
    for c in range(NCORES):
        b0 = c * BL
        xs = x[:, b0 : b0 + BL, :]
        # per-chain contiguous blocks, each column duplicated:
        # cols b*2S + 2t, +2t+1 = x[t, b0+b, :]; split into time-halves
        xtc = np.concatenate(
            [np.repeat(xs.transpose(2, 1, 0).reshape(IN, BL * S), 2, axis=1),
             np.ones((1, 2 * S * BL), np.float32)], axis=0
        ).reshape(IN + 1, BL, 2, S)  # [row, b, half, dup-cols-of-half]
        bxta = np.zeros((128, BL * S), np.float32)
        bxta[0 : IN + 1] = xtc[:, :, 0, :].reshape(IN + 1, BL * S)
        bxtb = np.zeros((128, BL * S), np.float32)
        bxtb[0 : IN + 1] = xtc[:, :, 1, :].reshape(IN + 1, BL * S)
        bwih_c = weights["bwih"].copy()
        h0off = BLOB_WIH_OFF["h0c"]
        bwih_c[0:H, h0off : h0off + 2 * BL] = np.repeat(
            hidden[0, b0 : b0 + BL, :].T, 2, axis=1
        )
        in_maps.append({
            "bwhh": weights["bwhh"],
            "bwih": bwih_c,
            "bcold": weights["bcold"],
            "bf": weights["bf"],
            "bxta": bxta,
            "bxtb": bxtb,
        })
    return in_maps


def postprocess(results):
    outs = []
    for r in results:
        # out layout [i, (b, f), j] -> (S*S, BL, 2)
        a = np.asarray(r["out"], np.float32).transpose(0, 2, 1).reshape(
            S * S, BL, 2
        )
        outs.append(np.ascontiguousarray(a))
    return np.concatenate(outs, axis=1)


_NC_CACHE = {}


def get_nc():
    if "nc" not in _NC_CACHE:
        _NC_CACHE["nc"] = build_nc()
    return _NC_CACHE["nc"]


LAST_RESULTS = None


def kernel(x, hidden, W_ih, W_hh, b_ih, b_hh, W1, b1, W2, b2, W3, b3, Wt, bt,
           _run_kwargs=None):
    global LAST_RESULTS
    weights = prep_weights(W_ih, W_hh, b_ih, b_hh, W1, b1, W2, b2, W3, b3, Wt, bt)
    in_maps = make_in_maps(x, hidden, weights)
    nc = get_nc()
    res = run_bass_kernel_spmd(
        nc, in_maps, core_ids=list(range(NCORES)), **(_run_kwargs or {})
    )
    LAST_RESULTS = res
    return postprocess(res.results)

